# revision 1
# baseline (speedup 1.0000x reference)
"""LocationMemoryBank retrieval kernel for 8 Trainium2 NeuronCores.

Strategy (v2): shard the memory table by location id across the 8 cores
(core c owns locs [c*1250, (c+1)*1250)). Queries are routed host-side to the
owning core and deduplicated: each core computes one weighted window-sum per
*unique* location hit (~8k unique of 16k queries => ~2x less gather traffic),
writing a compact [Urows, 512] result table. The final per-query expansion
(gather of result rows) is the host-side unshard step.

Device per 128-loc tile: two indirect DMAs gather each loc's 8-slot recent
window as two contiguous 4-slot chunks (one descriptor per partition;
partition p holds half-window p%2 of loc p//2). A block-diagonal weight
matrix is built on the DVE and the weighted sum over the 8 slots is done as
8 PE matmuls accumulating into one PSUM bank per tile.

indirect_dma_start HW semantics (probed): one descriptor per partition of the
offset AP; descriptor p copies the dest AP's free extent contiguously from
source row idx[p, 0].
"""

import os
import sys

import numpy as np

sys.path.insert(0, "/opt/trn_rl_repo")

L, M, D, B = 10000, 20, 512, 16384
K_RECENT = 8
N_CORES = 8
LPC = L // N_CORES          # locations per core
HALF = 4 * D                # one 4-slot half-window, in elements

_compiled = {}


def _build_bass(T_u):
    import concourse.bacc as bacc
    import concourse.bass as bass
    import concourse.mybir as mybir
    import concourse.tile as tile

    f32 = mybir.dt.float32
    i32 = mybir.dt.int32

    nc = bacc.Bacc(None)
    mem = nc.declare_dram_parameter("mem", [LPC * M, D], f32, isOutput=False)
    # idxs[t, p, s]: local flat slot index of the 4-slot chunk for call s
    idxs = nc.declare_dram_parameter("idxs", [128, T_u * 2], i32, isOutput=False)
    # wts[t, p, 4*s+j]: weight of slot 4*(p%2)+j of loc-rank t*128+64*s+p//2
    wts = nc.declare_dram_parameter("wts", [128, T_u * 8], f32, isOutput=False)
    # masks[p, s*128+m] = 1 if m == 64*s + p//2
    masks = nc.declare_dram_parameter("masks", [128, 256], f32, isOutput=False)
    out = nc.declare_dram_parameter("out", [T_u * 128, D], f32, isOutput=True)

    with tile.TileContext(nc) as tc:
        with (
            tc.tile_pool(name="const", bufs=1) as cpool,
            tc.tile_pool(name="gath", bufs=4) as gpool,
            tc.tile_pool(name="bd", bufs=3) as bdpool,
            tc.tile_pool(name="out", bufs=3) as opool,
            tc.tile_pool(name="psum", bufs=4, space="PSUM") as ppool,
        ):
            mask_t = cpool.tile([128, 256], f32)
            nc.sync.dma_start(out=mask_t[:], in_=masks[:])
            idx_all = cpool.tile([128, T_u * 2], i32)
            nc.sync.dma_start(out=idx_all[:], in_=idxs[:])
            w_all = cpool.tile([128, T_u * 8], f32)
            nc.sync.dma_start(out=w_all[:], in_=wts[:])

            for t in range(T_u):
                g_t = gpool.tile([128, 2 * HALF], f32)
                for s in range(2):
                    nc.gpsimd.indirect_dma_start(
                        out=g_t[:, s * HALF : (s + 1) * HALF],
                        out_offset=None,
                        in_=mem[:],
                        in_offset=bass.IndirectOffsetOnAxis(
                            ap=idx_all[:, 2 * t + s : 2 * t + s + 1], axis=0
                        ),
                    )

                ps = ppool.tile([128, D], f32, space="PSUM")
                for s in range(2):
                    for j in range(4):
                        g8 = 4 * s + j
                        bd = bdpool.tile([128, 128], f32)
                        nc.vector.tensor_scalar_mul(
                            bd[:],
                            mask_t[:, s * 128 : (s + 1) * 128],
                            w_all[:, 8 * t + g8 : 8 * t + g8 + 1],
                        )
                        nc.tensor.matmul(
                            out=ps[:],
                            lhsT=bd[:],
                            rhs=g_t[:, (s * 4 + j) * D : (s * 4 + j + 1) * D],
                            start=(g8 == 0),
                            stop=(g8 == 7),
                        )

                o_t = opool.tile([128, D], f32)
                nc.vector.tensor_copy(out=o_t[:], in_=ps[:])
                nc.sync.dma_start(out=out[t * 128 : (t + 1) * 128, :], in_=o_t[:])

    nc.finalize()
    return nc


def _get_bass(T_u):
    key = ("nc", T_u)
    if key not in _compiled:
        _compiled[key] = _build_bass(T_u)
    return _compiled[key]


def _host_prep(counts, loc_idx):
    """Route queries to owning shards, dedup by location, pack device inputs."""
    owner = (loc_idx // LPC).astype(np.int64)              # [B]

    wtab = np.zeros((K_RECENT + 1, K_RECENT), dtype=np.float64)
    for kk in range(1, K_RECENT + 1):
        e = np.exp(np.arange(kk, dtype=np.float64))
        wtab[kk, :kk] = e / e.sum()
    wtab = wtab.astype(np.float32)

    rank_q = np.zeros(B, dtype=np.int64)
    locs_all, n_uniq = [], []
    for c in range(N_CORES):
        sel = np.nonzero(owner == c)[0]
        locs, inv = np.unique(loc_idx[sel], return_inverse=True)
        rank_q[sel] = inv
        locs_all.append(locs)
        n_uniq.append(len(locs))
    T_u = max(1, -(-max(n_uniq) // 128))
    urows = T_u * 128

    # packing: tile t, call s, partition p -> loc rank r = t*128 + 64*s + p//2,
    # half h = p%2 covering slots [4h, 4h+4)
    p = np.arange(128)
    q_l = 64 * np.arange(2)[None, :] + (p[:, None] // 2)    # [128, 2]
    h = (p % 2)[:, None]                                    # [128, 1]

    idxs_all, wts_all = [], []
    for c in range(N_CORES):
        locs = locs_all[c]
        cl = counts[locs].astype(np.int64)
        kl = np.minimum(cl, K_RECENT)
        st = cl - kl
        ssl = np.zeros(urows, dtype=np.int64)
        ssl[: len(locs)] = (locs.astype(np.int64) - c * LPC) * M + st
        wl = np.zeros((urows, K_RECENT), dtype=np.float32)
        wl[: len(locs)] = wtab[kl]

        ss = ssl.reshape(T_u, 128)
        ww = wl.reshape(T_u, 128, K_RECENT)
        idx_pk = (ss[:, q_l] + 4 * h[None]).astype(np.int32)          # [T,128,2]
        w_pk = np.empty((T_u, 128, 8), dtype=np.float32)
        for s in range(2):
            for j in range(4):
                w_pk[:, :, 4 * s + j] = ww[:, q_l[:, s], (4 * h[:, 0] + j)]
        # partition-major for one-shot prefetch: [128, T*2], [128, T*8]
        idxs_all.append(np.ascontiguousarray(idx_pk.transpose(1, 0, 2).reshape(128, T_u * 2)))
        wts_all.append(np.ascontiguousarray(w_pk.transpose(1, 0, 2).reshape(128, T_u * 8)))

    mask = np.zeros((128, 256), dtype=np.float32)
    for s in range(2):
        mask[p, s * 128 + 64 * s + p // 2] = 1.0

    return idxs_all, wts_all, mask, T_u, owner, rank_q


def kernel(memory_feats, counts, loc_idx):
    from concourse.bass_utils import run_bass_kernel_spmd

    memory_feats = np.ascontiguousarray(memory_feats, dtype=np.float32)
    counts = np.asarray(counts, dtype=np.int32)
    loc_idx = np.asarray(loc_idx, dtype=np.int32)

    idxs_all, wts_all, mask, T_u, owner, rank_q = _host_prep(counts, loc_idx)
    nc = _get_bass(T_u)

    in_maps = [
        {
            "mem": memory_feats[c * LPC : (c + 1) * LPC].reshape(LPC * M, D),
            "idxs": idxs_all[c],
            "wts": wts_all[c],
            "masks": mask,
        }
        for c in range(N_CORES)
    ]
    trace = bool(int(os.environ.get("KERNEL_TRACE", "0")))
    res = run_bass_kernel_spmd(nc, in_maps, list(range(N_CORES)), trace=trace)
    _compiled["last_results"] = res
    res_stack = np.stack([res.results[c]["out"] for c in range(N_CORES)])
    return np.ascontiguousarray(res_stack[owner, rank_q])



# revision 2
# speedup vs baseline: 1.2820x; 1.2820x over previous
"""LocationMemoryBank retrieval kernel for 8 Trainium2 NeuronCores.

Strategy (v3): shard the memory table by location id across the 8 cores
(core c owns locs [c*1250, (c+1)*1250)). Queries are routed host-side to the
owning core and deduplicated; each core computes one weighted window-sum per
unique location hit, writing a compact [Urows, 512] result table. The final
per-query expansion (gather of result rows) is the host-side unshard step.

Device layout: one location per SBUF partition. Per 128-loc tile one indirect
DMA gathers each loc's 8-slot recent window as one contiguous 16KB descriptor
per partition (partition p holds all 8 slots of loc t*128+p). The weighted
sum over the 8 slots runs entirely on the DVE as a chain of fused
multiply-adds (scalar_tensor_tensor: acc = g_j * w_j + acc) with per-partition
scalar weights; no PE/PSUM involved. Unique locs are sorted by k=min(count,8)
descending so trailing slots of trailing tiles have all-zero weights and
their DVE ops are skipped.
"""

import os
import sys

import numpy as np

sys.path.insert(0, "/opt/trn_rl_repo")

L, M, D, B = 10000, 20, 512, 16384
K_RECENT = 8
N_CORES = 8
LPC = L // N_CORES          # locations per core
WIN = K_RECENT * D          # 8-slot window, in elements

_compiled = {}


def _build_bass(T_u, kmaxs):
    import concourse.bacc as bacc
    import concourse.bass as bass
    import concourse.mybir as mybir
    import concourse.tile as tile

    f32 = mybir.dt.float32
    i32 = mybir.dt.int32
    mult = mybir.AluOpType.mult
    add = mybir.AluOpType.add

    nc = bacc.Bacc(None)
    mem = nc.declare_dram_parameter("mem", [LPC * M, D], f32, isOutput=False)
    # idxs[p, t]: flat slot index of the window start for loc t*128+p
    idxs = nc.declare_dram_parameter("idxs", [128, T_u], i32, isOutput=False)
    # wts[p, 8*t+j]: weight of window slot j of loc t*128+p (0 if unused)
    wts = nc.declare_dram_parameter("wts", [128, T_u * 8], f32, isOutput=False)
    out = nc.declare_dram_parameter("out", [T_u * 128, D], f32, isOutput=True)

    with tile.TileContext(nc) as tc:
        with (
            tc.tile_pool(name="const", bufs=1) as cpool,
            tc.tile_pool(name="gath", bufs=3) as gpool,
            tc.tile_pool(name="out", bufs=3) as opool,
        ):
            idx_all = cpool.tile([128, T_u], i32)
            nc.sync.dma_start(out=idx_all[:], in_=idxs[:])
            w_all = cpool.tile([128, T_u * 8], f32)
            nc.sync.dma_start(out=w_all[:], in_=wts[:])

            for t in range(T_u):
                g_t = gpool.tile([128, WIN], f32)
                nc.gpsimd.indirect_dma_start(
                    out=g_t[:],
                    out_offset=None,
                    in_=mem[:],
                    in_offset=bass.IndirectOffsetOnAxis(
                        ap=idx_all[:, t : t + 1], axis=0
                    ),
                )

                acc = opool.tile([128, D], f32)
                kmax = kmaxs[t]
                if kmax == 0:
                    nc.vector.memset(acc[:], 0.0)
                else:
                    nc.vector.tensor_scalar_mul(
                        acc[:], g_t[:, 0:D], w_all[:, 8 * t : 8 * t + 1]
                    )
                    for j in range(1, kmax):
                        nc.vector.scalar_tensor_tensor(
                            out=acc[:],
                            in0=g_t[:, j * D : (j + 1) * D],
                            scalar=w_all[:, 8 * t + j : 8 * t + j + 1],
                            in1=acc[:],
                            op0=mult,
                            op1=add,
                        )
                nc.sync.dma_start(out=out[t * 128 : (t + 1) * 128, :], in_=acc[:])

    nc.finalize()
    return nc


def _get_bass(T_u, kmaxs):
    key = ("nc", T_u, tuple(kmaxs))
    if key not in _compiled:
        _compiled[key] = _build_bass(T_u, kmaxs)
    return _compiled[key]


def _host_prep(counts, loc_idx):
    """Route queries to owning shards, dedup + sort by k desc, pack inputs."""
    owner = (loc_idx // LPC).astype(np.int64)              # [B]

    wtab = np.zeros((K_RECENT + 1, K_RECENT), dtype=np.float64)
    for kk in range(1, K_RECENT + 1):
        e = np.exp(np.arange(kk, dtype=np.float64))
        wtab[kk, :kk] = e / e.sum()
    wtab = wtab.astype(np.float32)

    rank_q = np.zeros(B, dtype=np.int64)
    locs_all, n_uniq = [], []
    for c in range(N_CORES):
        sel = np.nonzero(owner == c)[0]
        locs, inv = np.unique(loc_idx[sel], return_inverse=True)
        kl = np.minimum(counts[locs].astype(np.int64), K_RECENT)
        # sort unique locs by k descending (stable: ties keep loc order)
        order = np.argsort(-kl, kind="stable")
        rank_of = np.empty(len(locs), dtype=np.int64)
        rank_of[order] = np.arange(len(locs))
        rank_q[sel] = rank_of[inv]
        locs_all.append(locs[order])
        n_uniq.append(len(locs))
    T_u = max(1, -(-max(n_uniq) // 128))
    urows = T_u * 128

    idxs_all, wts_all = [], []
    kmaxs = np.zeros(T_u, dtype=np.int64)
    for c in range(N_CORES):
        locs = locs_all[c]
        cl = counts[locs].astype(np.int64)
        kl = np.minimum(cl, K_RECENT)
        st = cl - kl
        ssl = np.zeros(urows, dtype=np.int64)
        ssl[: len(locs)] = (locs.astype(np.int64) - c * LPC) * M + st
        wl = np.zeros((urows, K_RECENT), dtype=np.float32)
        wl[: len(locs)] = wtab[kl]

        # row r = t*128 + p -> idx[p, t], wts[p, 8t+j]
        ss = ssl.reshape(T_u, 128).T                         # [128, T_u]
        ww = wl.reshape(T_u, 128, K_RECENT).transpose(1, 0, 2)  # [128, T_u, 8]
        idxs_all.append(np.ascontiguousarray(ss.astype(np.int32)))
        wts_all.append(np.ascontiguousarray(ww.reshape(128, T_u * 8)))

        km = np.zeros(urows, dtype=np.int64)
        km[: len(locs)] = kl
        kmaxs = np.maximum(kmaxs, km.reshape(T_u, 128).max(axis=1))

    return idxs_all, wts_all, [int(k) for k in kmaxs], T_u, owner, rank_q


def kernel(memory_feats, counts, loc_idx):
    from concourse.bass_utils import run_bass_kernel_spmd

    memory_feats = np.ascontiguousarray(memory_feats, dtype=np.float32)
    counts = np.asarray(counts, dtype=np.int32)
    loc_idx = np.asarray(loc_idx, dtype=np.int32)

    idxs_all, wts_all, kmaxs, T_u, owner, rank_q = _host_prep(counts, loc_idx)
    nc = _get_bass(T_u, kmaxs)

    in_maps = [
        {
            "mem": memory_feats[c * LPC : (c + 1) * LPC].reshape(LPC * M, D),
            "idxs": idxs_all[c],
            "wts": wts_all[c],
        }
        for c in range(N_CORES)
    ]
    trace = bool(int(os.environ.get("KERNEL_TRACE", "0")))
    res = run_bass_kernel_spmd(nc, in_maps, list(range(N_CORES)), trace=trace)
    _compiled["last_results"] = res
    res_stack = np.stack([res.results[c]["out"] for c in range(N_CORES)])
    return np.ascontiguousarray(res_stack[owner, rank_q])


# revision 3
# speedup vs baseline: 1.3909x; 1.0849x over previous
"""LocationMemoryBank retrieval kernel for 8 Trainium2 NeuronCores.

Strategy (v3): shard the memory table by location id across the 8 cores
(core c owns locs [c*1250, (c+1)*1250)). Queries are routed host-side to the
owning core and deduplicated; each core computes one weighted window-sum per
unique location hit, writing a compact [Urows, 512] result table. The final
per-query expansion (gather of result rows) is the host-side unshard step.

Device layout: one location per SBUF partition. Per 128-loc tile one indirect
DMA gathers each loc's 8-slot recent window as one contiguous 16KB descriptor
per partition (partition p holds all 8 slots of loc t*128+p). The weighted
sum over the 8 slots runs entirely on the DVE as a chain of fused
multiply-adds (scalar_tensor_tensor: acc = g_j * w_j + acc) with per-partition
scalar weights; no PE/PSUM involved. Unique locs are sorted by k=min(count,8)
descending so trailing slots of trailing tiles have all-zero weights and
their DVE ops are skipped.
"""

import os
import sys

import numpy as np

sys.path.insert(0, "/opt/trn_rl_repo")

L, M, D, B = 10000, 20, 512, 16384
K_RECENT = 8
N_CORES = 8
LPC = L // N_CORES          # locations per core
WIN = K_RECENT * D          # 8-slot window, in elements

_compiled = {}


def _build_bass(T_u, kmaxs):
    import concourse.bacc as bacc
    import concourse.bass as bass
    import concourse.mybir as mybir
    import concourse.tile as tile

    f32 = mybir.dt.float32
    i32 = mybir.dt.int32
    mult = mybir.AluOpType.mult
    add = mybir.AluOpType.add

    nc = bacc.Bacc(None)
    mem = nc.declare_dram_parameter("mem", [LPC * M, D], f32, isOutput=False)
    # idxs[p, t]: flat slot index of the window start for loc t*128+p
    idxs = nc.declare_dram_parameter("idxs", [128, T_u], i32, isOutput=False)
    # wts[p, 8*t+j]: weight of window slot j of loc t*128+p (0 if unused)
    wts = nc.declare_dram_parameter("wts", [128, T_u * 8], f32, isOutput=False)
    out = nc.declare_dram_parameter("out", [T_u * 128, D], f32, isOutput=True)

    with tile.TileContext(nc) as tc:
        with (
            tc.tile_pool(name="const", bufs=1) as cpool,
            tc.tile_pool(name="gath", bufs=3) as gpool,
            tc.tile_pool(name="out", bufs=3) as opool,
        ):
            idx_all = cpool.tile([128, T_u], i32)
            nc.sync.dma_start(out=idx_all[:], in_=idxs[:])
            w_all = cpool.tile([128, T_u * 8], f32)
            nc.sync.dma_start(out=w_all[:], in_=wts[:])

            for t in range(T_u):
                kmax = kmaxs[t]
                if kmax > 0:
                    # gather only the first kmax window slots of each loc
                    g_t = gpool.tile([128, kmax * D], f32)
                    nc.gpsimd.indirect_dma_start(
                        out=g_t[:],
                        out_offset=None,
                        in_=mem[:],
                        in_offset=bass.IndirectOffsetOnAxis(
                            ap=idx_all[:, t : t + 1], axis=0
                        ),
                    )

                acc = opool.tile([128, D], f32)
                if kmax == 0:
                    nc.vector.memset(acc[:], 0.0)
                else:
                    nc.vector.tensor_scalar_mul(
                        acc[:], g_t[:, 0:D], w_all[:, 8 * t : 8 * t + 1]
                    )
                    for j in range(1, kmax):
                        nc.vector.scalar_tensor_tensor(
                            out=acc[:],
                            in0=g_t[:, j * D : (j + 1) * D],
                            scalar=w_all[:, 8 * t + j : 8 * t + j + 1],
                            in1=acc[:],
                            op0=mult,
                            op1=add,
                        )
                nc.sync.dma_start(out=out[t * 128 : (t + 1) * 128, :], in_=acc[:])

    nc.finalize()
    return nc


def _get_bass(T_u, kmaxs):
    key = ("nc", T_u, tuple(kmaxs))
    if key not in _compiled:
        _compiled[key] = _build_bass(T_u, kmaxs)
    return _compiled[key]


def _host_prep(counts, loc_idx):
    """Route queries to owning shards, dedup + sort by k desc, pack inputs."""
    owner = (loc_idx // LPC).astype(np.int64)              # [B]

    wtab = np.zeros((K_RECENT + 1, K_RECENT), dtype=np.float64)
    for kk in range(1, K_RECENT + 1):
        e = np.exp(np.arange(kk, dtype=np.float64))
        wtab[kk, :kk] = e / e.sum()
    wtab = wtab.astype(np.float32)

    rank_q = np.zeros(B, dtype=np.int64)
    locs_all, n_uniq = [], []
    for c in range(N_CORES):
        sel = np.nonzero(owner == c)[0]
        locs, inv = np.unique(loc_idx[sel], return_inverse=True)
        kl = np.minimum(counts[locs].astype(np.int64), K_RECENT)
        # sort unique locs by k descending (stable: ties keep loc order)
        order = np.argsort(-kl, kind="stable")
        rank_of = np.empty(len(locs), dtype=np.int64)
        rank_of[order] = np.arange(len(locs))
        rank_q[sel] = rank_of[inv]
        locs_all.append(locs[order])
        n_uniq.append(len(locs))
    T_u = max(1, -(-max(n_uniq) // 128))
    urows = T_u * 128

    idxs_all, wts_all = [], []
    kmaxs = np.zeros(T_u, dtype=np.int64)
    for c in range(N_CORES):
        locs = locs_all[c]
        cl = counts[locs].astype(np.int64)
        kl = np.minimum(cl, K_RECENT)
        st = cl - kl
        ssl = np.zeros(urows, dtype=np.int64)
        ssl[: len(locs)] = (locs.astype(np.int64) - c * LPC) * M + st
        wl = np.zeros((urows, K_RECENT), dtype=np.float32)
        wl[: len(locs)] = wtab[kl]

        # row r = t*128 + p -> idx[p, t], wts[p, 8t+j]
        ss = ssl.reshape(T_u, 128).T                         # [128, T_u]
        ww = wl.reshape(T_u, 128, K_RECENT).transpose(1, 0, 2)  # [128, T_u, 8]
        idxs_all.append(np.ascontiguousarray(ss.astype(np.int32)))
        wts_all.append(np.ascontiguousarray(ww.reshape(128, T_u * 8)))

        km = np.zeros(urows, dtype=np.int64)
        km[: len(locs)] = kl
        kmaxs = np.maximum(kmaxs, km.reshape(T_u, 128).max(axis=1))

    return idxs_all, wts_all, [int(k) for k in kmaxs], T_u, owner, rank_q


def kernel(memory_feats, counts, loc_idx):
    from concourse.bass_utils import run_bass_kernel_spmd

    memory_feats = np.ascontiguousarray(memory_feats, dtype=np.float32)
    counts = np.asarray(counts, dtype=np.int32)
    loc_idx = np.asarray(loc_idx, dtype=np.int32)

    idxs_all, wts_all, kmaxs, T_u, owner, rank_q = _host_prep(counts, loc_idx)
    nc = _get_bass(T_u, kmaxs)

    in_maps = [
        {
            "mem": memory_feats[c * LPC : (c + 1) * LPC].reshape(LPC * M, D),
            "idxs": idxs_all[c],
            "wts": wts_all[c],
        }
        for c in range(N_CORES)
    ]
    trace = bool(int(os.environ.get("KERNEL_TRACE", "0")))
    res = run_bass_kernel_spmd(nc, in_maps, list(range(N_CORES)), trace=trace)
    _compiled["last_results"] = res
    res_stack = np.stack([res.results[c]["out"] for c in range(N_CORES)])
    return np.ascontiguousarray(res_stack[owner, rank_q])
